# revision 29
# baseline (speedup 1.0000x reference)
"""Causal multi-head attention (16 heads, hd=64) on 8 trn2 NeuronCores.

Sharding: core c -> batch b = c // 4, head-group g = c % 4 (4 heads = 256
columns of Wq/Wk/Wv).  Each core computes its [S, 256] slice of the three
outputs (attn out, K_cache, V_cache); the host gathers slices.

Per-core pipeline (Tile framework), all matmuls in bf16 (f32 PSUM
accumulation, ~4e-3 relative error vs the f32 reference):
  - xT [1024, S] is host-transposed x[b]; weights/biases host-sliced.
  - KT/QT [c, q] computed directly (lhsT = W chunk), per-partition bias
    added during the DVE eviction; K_cache leaves the chip in kt's [c, s]
    layout (contiguous DMA) and the host transposes it in the gather.
  - Vf natural [s, c] (rank-1 bias matmul) -> V_cache + V_aug tiles
    [k, 65] per head (ones column -> softmax denominator; ones written
    by DVE -- a strided sub-word DMA would RMW-race adjacent columns).
  - scores ST[k, q]: the two heads of a pair run as concurrent
    row-tiled matmuls (K=64, partition offsets 0/64) into one
    [128, 1024] psum tile; diagonal blocks narrowed to the valid q
    range; one exp per k-tile over both heads (ACT, scale=1/8, per-k
    pad bias), fill-0 affine_select on the 128-wide partial triangle.
  - AV: out_unnorm[65, q] += V_aug.T @ PT over k-tiles; normalize in
    [d, q] layout: reciprocal of the ones-row sum, partition-broadcast
    (gpsimd DMA), one DVE multiply; `out` leaves the chip [c, s] and
    the host transposes it in the gather, like kct.
  - Emission order per q-slice: Q-projection of pair 0 first, then the
    full (non-diagonal) attention tiles -- they only need old kt/va --
    with the K projection, pair-1 Q and the V wave emitted behind them
    as PE filler while ACT chews the exp backlog; diagonal tiles and
    pair 1 follow.  The scalar engine then never idles more than the
    ~1.7us pair-0 Q projection at a slice boundary.
  - Inputs land on three DMA queues (x halves on sync+scalar, weights
    on gpsimd, wq first to match the Q-first emission); vc leaves on
    the gpsimd queue so mid-kernel stores never queue behind x loads.
  - PE warm-up runs off a gpsimd-memset tile so it starts ~6.3us in,
    independent of any DMA; heater matmuls inside the last (ACT-bound)
    pair keep HAM from re-throttling the PE near the end.
"""

import numpy as np

P = 128
S = 2048
HIN = 1024
C = 256  # columns per core = 4 heads * 64
HD = 64
NCORES = 8
HC = HIN // P  # 8 contraction chunks
NKT = S // P  # 16 k-tiles
QW = 512  # q-slice width
NQ = S // QW  # 4 q-slices
NPAIR = C // P  # 2 head-pairs per core

_nc_cache = None


def build_nc():
    import concourse.bacc as bacc
    import concourse.mybir as mybir
    from concourse.tile import TileContext
    from contextlib import ExitStack

    f32 = mybir.dt.float32
    f32r = mybir.dt.float32r
    bf16 = mybir.dt.bfloat16
    Exp = mybir.ActivationFunctionType.Exp

    nc = bacc.Bacc(None, target_bir_lowering=False)

    # x and weights arrive partition-major (host pre-arranged) so every
    # DMA descriptor moves a contiguous 4KB run per partition
    xt = nc.declare_dram_parameter("xt", [4, 2, P, (HC // 2) * (S // 4)], bf16,
                                   isOutput=False)
    wq = nc.declare_dram_parameter("wq", [P, HC * C], bf16, isOutput=False)
    wk = nc.declare_dram_parameter("wk", [P, HC * C], bf16, isOutput=False)
    wv = nc.declare_dram_parameter("wv", [P, HC * C], bf16, isOutput=False)
    # small per-partition constants are PACKED into two tensors (f32:
    # bqc|bkc|padneg, bf16: tri|ones) -- 7 fragmented 8-64B/partition
    # DMAs measured ~10GB/s on the scalar HWDGE queue and gated the
    # first exp/AV by several us
    cpkf = nc.declare_dram_parameter("cpkf", [P, 2 * NPAIR + NKT], f32,
                                     isOutput=False)
    cpkb = nc.declare_dram_parameter("cpkb", [P, 2 * P + C], bf16,
                                     isOutput=False)
    bv = nc.declare_dram_parameter("bv", [1, C], bf16, isOutput=False)
    onesr = nc.declare_dram_parameter("onesr", [1, HD], f32r, isOutput=False)
    out = nc.declare_dram_parameter("out", [C, S], bf16, isOutput=True)
    kct = nc.declare_dram_parameter("kct", [C, S], bf16, isOutput=True)
    vc = nc.declare_dram_parameter("vc", [S, C], bf16, isOutput=True)

    with TileContext(nc) as tc, ExitStack() as ctx:
        persist = ctx.enter_context(tc.tile_pool(name="persist", bufs=1))
        xt_sb = persist.tile([P, HC, S], bf16)
        wq_sb = persist.tile([P, HC, C], bf16)
        wk_sb = persist.tile([P, HC, C], bf16)
        wv_sb = persist.tile([P, HC, C], bf16)
        cpkf_sb = persist.tile([P, 2 * NPAIR + NKT], f32)
        bqc_sb = cpkf_sb[:, 0:NPAIR]
        bkc_sb = cpkf_sb[:, NPAIR : 2 * NPAIR]
        pn_sb = cpkf_sb[:, 2 * NPAIR : 2 * NPAIR + NKT]
        cpkb_sb = persist.tile([P, 2 * P + C], bf16)
        tri_sb = cpkb_sb[:, 0 : 2 * P].rearrange("p (h b) -> p h b", h=2)
        ones_sb = cpkb_sb[:, 2 * P : 2 * P + C]
        bv_sb = persist.tile([1, C], bf16)
        warm_sb = persist.tile([HD, NKT], f32)
        ones_r = persist.tile([1, HD], f32r)
        qt_bf = persist.tile([P, NPAIR, S], bf16)
        kt_sb = persist.tile([P, NPAIR, S], bf16)
        va_bf = persist.tile([P, NKT, NPAIR, 2 * (HD + 1)], bf16)
        heat_sb = persist.tile([P, P + QW], bf16)

        # the heater tile is DVE-memset (vector idles at startup), so the
        # PE warm-up has no DMA or gpsimd dependency at all
        nc.vector.memset(heat_sb[:], 0)
        quarter = S // 4
        half = HC // 2

        def x_piece(eng, h, g):
            eng.dma_start(
                xt_sb[:, g * half : (g + 1) * half,
                      h * quarter : (h + 1) * quarter],
                xt[h, g, :, :].rearrange("p (a s) -> p a s", a=half),
            )

        # packed constants on the scalar queue (each dma_start costs
        # ~0.65us of descriptor-gen on its issuing engine), then the g=1
        # half of quarter 0, wv, and quarters 2-3; sync carries quarter-0
        # g=0 and quarter 1 (it later carries out/kct), gpsimd the wk/wq
        # halves (it later carries vc) -- three queues in parallel, with
        # everything qi=0 needs (quarter 0, wk, wq, wv) landed by ~17us
        nc.scalar.dma_start(cpkf_sb[:], cpkf[:])
        nc.scalar.dma_start(cpkb_sb[:], cpkb[:])
        nc.scalar.dma_start(bv_sb[:], bv[:])
        nc.scalar.dma_start(ones_r[:], onesr[:])
        nc.scalar.dma_start(
            wv_sb[:, :, :], wv[:, :].rearrange("p (a c) -> p a c", a=HC)
        )
        # wk lands first (the qi=0 emission is K-first: everything at
        # qi=0 needs kt), split in chunk-halves so the j0-3 projections
        # start as soon as the first half arrives
        for w_sb, w in ((wk_sb, wk), (wq_sb, wq)):
            for g in range(2):
                nc.gpsimd.dma_start(
                    w_sb[:, g * half : (g + 1) * half, :],
                    w[:, g * half * C : (g + 1) * half * C]
                    .rearrange("p (a c) -> p a c", a=half),
                )
        # all of x on the (fast) sync queue, quarter-major: the scalar
        # HWDGE queue measured ~3-4x slower, so it only carries the small
        # consts and wv
        for h in range(4):
            for g in range(2):
                x_piece(nc.sync, h, g)
        # dummy broadcast: loads the gpsimd PartitionBroadcast ucode
        # library (~7us) during the startup DMA wait instead of at the
        # first normalize
        nc.gpsimd.partition_broadcast(warm_sb[:], pn_sb[0:1, :])
        # ones columns of V_aug (positions 64 and 129).  Written by DVE, not
        # DMA: a strided sub-word DMA write would RMW-race the adjacent
        # DVE-written V columns.
        ones3 = ones_sb[:, : NKT * NPAIR].rearrange("p (a b) -> p a b", a=NKT)
        nc.vector.tensor_copy(
            out=va_bf[:, :, :, HD : HD + 1], in_=ones3[:, :, :, None]
        )
        nc.vector.tensor_copy(
            out=va_bf[:, :, :, 2 * HD + 1 : 2 * HD + 2], in_=ones3[:, :, :, None]
        )

        psum = ctx.enter_context(tc.tile_pool(name="psum", bufs=2, space="PSUM"))
        work = ctx.enter_context(tc.tile_pool(name="work", bufs=3))

        # p-state warm-up: the PE reaches full clock during the input DMA
        # wait; the results are never read.  The upfront heaters write
        # the av-pool slot (first real AV write is ~22us in), NOT the
        # proj pool -- a proj slot target would serialize the projection
        # chains against every heater the scheduler spreads into a
        # DMA-wait gap, and an st slot would halve the scores->exp
        # pipeline depth.  The late (qi=3) heaters use a proj-pool slot,
        # idle by then.
        heat_ps = psum.tile([HD + 1, 2, QW], f32, tag="av", bufs=1,
                            name="heat_ps")
        warm_ps = psum.tile([P, QW], f32, tag="proj", bufs=2, name="warm_ps")

        def heater(n=1, late=False):
            for _ in range(n):
                if late:
                    nc.tensor.matmul(
                        warm_ps, heat_sb[:, 0:P], heat_sb[:, P : P + QW],
                        start=True, stop=True,
                    )
                else:
                    nc.tensor.matmul(
                        heat_ps[:, 0, :], heat_sb[:, 0 : HD + 1],
                        heat_sb[:, P : P + QW], start=True, stop=True,
                    )

        heater(26)

        def proj_slice(qi, w_sb, b_sb, dst, pairs=(0, 1), evict="vector"):
            # project one q/k-slice of K or Q for the given pairs; for
            # qi=0 the two x chunk-halves are in flight on separate
            # queues, so run j 0-3 before j 4-7 (PE consumes half 0
            # while half 1 lands).  evict="scalar" runs the bias
            # eviction on the (idle-at-phase-boundary) scalar engine
            # instead of queueing behind the DVE's normalize backlog.
            qsl = slice(qi * QW, (qi + 1) * QW)
            jws = ((0, 4), (4, 8)) if qi == 0 else ((0, 8),)
            for p in pairs:
                csl = slice(p * P, (p + 1) * P)
                ps = psum.tile([P, QW], f32, tag="proj", bufs=2, name="p_ps")
                for j0, j1 in jws:
                    for j in range(j0, j1):
                        nc.tensor.matmul(
                            ps, w_sb[:, j, csl], xt_sb[:, j, qsl],
                            start=(j == 0), stop=(j == HC - 1),
                        )
                if evict == "scalar":
                    nc.scalar.add(dst[:, p, qsl], ps, b_sb[:, p : p + 1])
                else:
                    nc.vector.tensor_scalar_add(
                        dst[:, p, qsl], ps, b_sb[:, p : p + 1]
                    )

        def v_tile(i):
            ksl = slice(i * P, (i + 1) * P)
            ps = psum.tile([P, QW], f32, tag="proj", bufs=2, name="v_ps")[:, :C]
            for j in range(HC):
                nc.tensor.matmul(
                    ps, xt_sb[:, j, ksl], wv_sb[:, j, :],
                    start=(j == 0), stop=False,
                )
            nc.tensor.matmul(
                ps, ones_sb[:1, :P], bv_sb[:1, :], start=False, stop=True
            )
            sb = work.tile([P, C], bf16, tag="projsb", bufs=4, name="v_sb")
            # vector, NOT nc.any: the scheduler puts "any" copies on
            # the scalar engine, where each blocks the following exps
            # in the ACT FIFO while it waits for its V matmuls
            # (gpsimd cannot read PSUM)
            nc.vector.tensor_copy(out=sb[:], in_=ps)
            # vc leaves on the gpsimd queue so it never queues behind
            # the x loads on sync
            nc.gpsimd.dma_start(vc[ksl, :], sb[:])
            for p in range(NPAIR):
                nc.vector.tensor_copy(
                    out=va_bf[:, i, p, 0:HD], in_=sb[:, p * P : p * P + HD]
                )
                nc.vector.tensor_copy(
                    out=va_bf[:, i, p, HD + 1 : 2 * HD + 1],
                    in_=sb[:, p * P + HD : (p + 1) * P],
                )

        def tile_scores_exp(qi, p, t):
            # scores for both heads (concurrent row-tiled matmuls), exp,
            # and the diagonal-block mask; returns the pt tile + width
            ksl = slice(t * P, (t + 1) * P)
            d = t - 4 * qi
            W = QW if d < 0 else QW - d * P
            q0 = qi * QW + (0 if d < 0 else d * P)
            st = psum.tile([P, 2 * QW], f32, tag="st", bufs=2, name="st")
            nc.tensor.matmul(
                st[:, 0:W], kt_sb[0:HD, p, ksl],
                qt_bf[0:HD, p, q0 : q0 + W], start=True, stop=True,
            )
            nc.tensor.matmul(
                st[:, QW : QW + W], kt_sb[HD:P, p, ksl],
                qt_bf[HD:P, p, q0 : q0 + W], start=True, stop=True,
            )
            pt = work.tile([P, 2, QW], bf16, tag="pt", bufs=28, name="pt")
            st3 = st[:].rearrange("p (h w) -> p h w", h=2)[:, :, 0:W]
            nc.scalar.activation(
                pt[:, :, 0:W], st3, Exp, bias=pn_sb[:, t : t + 1],
                scale=0.125,
            )
            if d >= 0:
                nc.vector.tensor_mul(
                    pt[:, :, 0:P], pt[:, :, 0:P], tri_sb[:]
                )
            return pt, W

        def tile_av(av, pt, W, p, t, start, stop):
            nc.tensor.matmul(
                av[:, 0, QW - W :], va_bf[:, t, p, 0 : HD + 1],
                pt[:, 0, 0:W], start=start, stop=stop,
            )
            nc.tensor.matmul(
                av[:, 1, QW - W :], va_bf[:, t, p, HD + 1 : 2 * HD + 2],
                pt[:, 1, 0:W], start=start, stop=stop,
            )

        def attention_core(qi, p, trange=None, avs=None, heat_every=0):
            # one [65, 2, QW] psum tile holds BOTH heads' unnormalized AV
            # (2 adjacent banks) so normalize can run single merged
            # [1, 2*QW] ops instead of per-head ones
            av = avs if avs is not None else psum.tile(
                [HD + 1, 2, QW], f32, tag="av", bufs=1, name="av"
            )
            tmax = 4 * qi + 4
            for ti, t in enumerate(trange if trange is not None else range(tmax)):
                if heat_every and ti and ti % heat_every == 0:
                    heater(late=True)
                pt, W = tile_scores_exp(qi, p, t)
                tile_av(av, pt, W, p, t, start=(t == 0), stop=(t == tmax - 1))
            return av

        def normalize(qi, p, av, use_pe_outer):
            # normalize in [d, q] layout: reciprocal of the ones-row sum
            # (both heads in one [1, 2*QW] sweep), partition-broadcast
            # (gpsimd DMA; PE outer product on the last slice so gpsimd's
            # slow post-broadcast drain doesn't stretch the tail), one
            # DVE multiply per head; `out` leaves the chip [c, s] (the
            # host transposes it in the gather, like kct).
            # The whole av tile is copied to SBUF in ONE op first: the
            # single av psum slot is WAR-shared with the next pair's
            # accumulation, and holding it through the ~5us
            # rcp->broadcast->mul latency chain cascades (via the pt
            # pool) into the next phase's exps.
            if not use_pe_outer:
                avc = work.tile([HD + 1, 2, QW], f32, tag="avc", bufs=2,
                                name="avc")
                nc.vector.tensor_copy(out=avc[:], in_=av[:])
                av = avc
            dsb = work.tile([1, 2 * QW], f32, tag="dsb", bufs=2, name="dsb")
            dsb3 = dsb[:].rearrange("p (h w) -> p h w", h=2)
            if use_pe_outer:
                nc.scalar.copy(dsb3, av[HD : HD + 1, :, :])
            else:
                nc.vector.tensor_copy(out=dsb3, in_=av[HD : HD + 1, :, :])
            rcp = work.tile([1, 2 * QW], f32, tag="rcp", bufs=2, name="rcp")
            nc.vector.reciprocal_approx_fast(rcp[:], dsb[:])
            rb_sb = work.tile([HD, 2 * QW], f32, tag="rb", bufs=2, name="rb")
            if not use_pe_outer:
                nc.gpsimd.partition_broadcast(rb_sb[:], rcp[:])
            else:
                rcp_r = work.tile([1, 2 * QW], f32r, tag="rcpr", bufs=2,
                                  name="rcpr")
                nc.vector.tensor_copy(out=rcp_r[:], in_=rcp[:])
                rb_ps = psum.tile([P, 2 * QW], f32, tag="st", bufs=2,
                                  name="rb_ps")[0:HD, :]
                for h in range(2):  # f32 moving operand caps at 512
                    nc.tensor.matmul(
                        rb_ps[:, h * QW : (h + 1) * QW], ones_r[:],
                        rcp_r[:, h * QW : (h + 1) * QW],
                        start=True, stop=True,
                    )
                nc.vector.tensor_copy(out=rb_sb[:], in_=rb_ps)
            rb3 = rb_sb[:].rearrange("p (h w) -> p h w", h=2)
            for h in range(2):
                osb = work.tile([HD, QW], bf16, tag="osb", bufs=3, name="osb")
                nc.vector.tensor_mul(osb[:], av[0:HD, h, :], rb3[:, h, :])
                col = p * P + h * HD
                nc.sync.dma_start(
                    out[col : col + HD, qi * QW : (qi + 1) * QW], osb[:]
                )

        # Q-first emission: the full (non-diagonal) attention tiles of a
        # slice only need the new Q projection (their kt/va slices are
        # older), so they start ~1.7us into the phase; the K projection,
        # pair-1 Q and the V wave trail behind them as PE filler while
        # the scalar engine drains the exp backlog.  Pair 1's normalize
        # is deferred past the next slice's Q projection, and the very
        # last pair normalizes via the PE outer product (the PE is idle
        # by then) so gpsimd's slow post-broadcast drain starts well
        # before the end.
        # Software-pipelined emission: ALL of a slice's scores+exp are
        # emitted before its PE fillers (V tiles, AV sweeps), buffered in
        # the deep pt pool, and the NEXT slice's pair-0 full scores+exp
        # are hoisted before this slice's pair-1 AV sweep -- the exp
        # stream on the scalar engine then never waits on filler blocks.
        # Pair-1 normalize is deferred past the next slice's head, and
        # the very last pair normalizes via the PE outer product (the PE
        # is idle by then) so gpsimd's slow post-broadcast drain starts
        # well before the end.
        pending = None
        full0_pts = []
        proj_slice(0, wk_sb, bkc_sb, kt_sb)
        proj_slice(0, wq_sb, bqc_sb, qt_bf, pairs=(0,))
        for qi in range(NQ):
            if pending is not None:
                normalize(qi - 1, 1, pending, use_pe_outer=False)
            av0 = psum.tile([HD + 1, 2, QW], f32, tag="av", bufs=1, name="av")
            for t, pt, W in full0_pts:
                tile_av(av0, pt, W, 0, t, start=(t == 0), stop=False)
            if qi > 0:
                proj_slice(qi, wk_sb, bkc_sb, kt_sb)
            proj_slice(qi, wq_sb, bqc_sb, qt_bf, pairs=(1,), evict="scalar")
            pts0 = []
            for t in range(4 * qi, 4 * qi + 4):
                pts0.append((t,) + tile_scores_exp(qi, 0, t))
            pts1 = []
            for t in range(4 * qi + 4):
                pts1.append((t,) + tile_scores_exp(qi, 1, t))
            for t, pt, W in pts0:
                v_tile(t)
                tile_av(av0, pt, W, 0, t,
                        start=(t == 0 and qi == 0),
                        stop=(t == 4 * qi + 3))
            normalize(qi, 0, av0, use_pe_outer=False)
            if qi < NQ - 1:
                # next slice's pair-0 full tiles: kt slices <= qi are all
                # projected, only the next qt is needed
                proj_slice(qi + 1, wq_sb, bqc_sb, qt_bf, pairs=(0,),
                           evict="scalar")
                full0_pts = []
                for t in range(4 * (qi + 1)):
                    full0_pts.append((t,) + tile_scores_exp(qi + 1, 0, t))
            av1 = psum.tile([HD + 1, 2, QW], f32, tag="av", bufs=1, name="av")
            for ti, (t, pt, W) in enumerate(pts1):
                if qi == NQ - 1 and ti:
                    heater(late=True)
                tile_av(av1, pt, W, 1, t,
                        start=(t == 0), stop=(t == 4 * qi + 3))
            pending = av1
            # K_cache leaves the chip in kt's [c, s] layout (contiguous
            # DMA); the host transposes it during the gather
            nc.sync.dma_start(
                kct[:, qi * QW : (qi + 1) * QW]
                .rearrange("(a p) s -> p a s", p=P),
                kt_sb[:, :, qi * QW : (qi + 1) * QW],
            )
        normalize(NQ - 1, 1, pending, use_pe_outer=True)

    nc.finalize()
    return nc


def get_nc():
    global _nc_cache
    if _nc_cache is None:
        _nc_cache = build_nc()
    return _nc_cache


def _w_pre(Wslice):
    # [HIN, C] -> partition-major [P, HC*C]: per partition p a contiguous
    # 4KB run holding its rows of every contraction chunk
    return np.ascontiguousarray(
        Wslice.reshape(HC, P, C).transpose(1, 0, 2).reshape(P, HC * C)
    )


def _x_pre(xT):
    # [HIN, S] -> [4(h), 2(g), P, 4*512]: per (quarter h, chunk-half g,
    # partition p) a contiguous 4KB run
    a = xT.reshape(2, 4, P, 4, QW)  # [g, jm, p, h, s]
    return np.ascontiguousarray(
        a.transpose(3, 0, 2, 1, 4).reshape(4, 2, P, 4 * QW)
    )


def make_in_maps(x, pad_mask, Wq, bq, Wk, bk, Wv, bv):
    from ml_dtypes import bfloat16

    x = np.asarray(x, np.float32)
    pad_mask = np.asarray(pad_mask, np.float32)
    Wq = np.asarray(Wq, bfloat16)
    bq = np.asarray(bq, np.float32)
    Wk = np.asarray(Wk, bfloat16)
    bk = np.asarray(bk, np.float32)
    Wv = np.asarray(Wv, bfloat16)
    bv = np.asarray(bv, bfloat16)
    xts = [_x_pre(x[b].T.astype(bfloat16)) for b in range(2)]
    in_maps = []
    for c in range(NCORES):
        b, g = divmod(c, 4)
        cols = slice(g * C, (g + 1) * C)
        pn = ((pad_mask[b] - 1.0) * 1e6).reshape(NKT, P).T  # [P, NKT]
        trih = (np.arange(P)[None, :] >= np.arange(P)[:, None]).astype(bfloat16)
        cpkf = np.concatenate(
            [bq[cols].reshape(NPAIR, P).T, bk[cols].reshape(NPAIR, P).T, pn],
            axis=1,
        ).astype(np.float32)
        cpkb = np.concatenate(
            [trih, trih, np.ones((P, C), bfloat16)], axis=1
        ).astype(bfloat16)
        in_maps.append(
            dict(
                xt=xts[b],
                cpkf=np.ascontiguousarray(cpkf),
                cpkb=np.ascontiguousarray(cpkb),
                onesr=np.ones((1, HD), np.float32),
                wq=_w_pre(Wq[:, cols]),
                wk=_w_pre(Wk[:, cols]),
                wv=_w_pre(Wv[:, cols]),
                bv=np.ascontiguousarray(bv[cols].reshape(1, C)),
            )
        )
    return in_maps


def gather(results):
    B = 2
    out = np.empty((B, S, HIN), np.float32)
    kcache = np.empty((B, S, HIN), np.float32)
    vcache = np.empty((B, S, HIN), np.float32)
    for c in range(NCORES):
        b, g = divmod(c, 4)
        cols = slice(g * C, (g + 1) * C)
        out[b, :, cols] = results[c]["out"].T
        kcache[b, :, cols] = results[c]["kct"].T
        vcache[b, :, cols] = results[c]["vc"]
    return out, kcache, vcache


def kernel(x, pad_mask, Wq, bq, Wk, bk, Wv, bv):
    from concourse.bass_utils import run_bass_kernel_spmd

    nc = get_nc()
    in_maps = make_in_maps(x, pad_mask, Wq, bq, Wk, bk, Wv, bv)
    res = run_bass_kernel_spmd(nc, in_maps, list(range(NCORES)))
    return gather(res.results)


# revision 31
# speedup vs baseline: 1.0061x; 1.0061x over previous
"""Causal multi-head attention (16 heads, hd=64) on 8 trn2 NeuronCores.

Sharding: core c -> batch b = c // 4, head-group g = c % 4 (4 heads = 256
columns of Wq/Wk/Wv).  Each core computes its [S, 256] slice of the three
outputs (attn out, K_cache, V_cache); the host gathers slices.

Per-core pipeline (Tile framework), all matmuls in bf16 (f32 PSUM
accumulation, ~4e-3 relative error vs the f32 reference):
  - xT [1024, S] is host-transposed x[b]; weights/biases host-sliced.
  - KT/QT [c, q] computed directly (lhsT = W chunk), per-partition bias
    added during the DVE eviction; K_cache leaves the chip in kt's [c, s]
    layout (contiguous DMA) and the host transposes it in the gather.
  - Vf natural [s, c] (rank-1 bias matmul) -> V_cache + V_aug tiles
    [k, 65] per head (ones column -> softmax denominator; ones written
    by DVE -- a strided sub-word DMA would RMW-race adjacent columns).
  - scores ST[k, q]: the two heads of a pair run as concurrent
    row-tiled matmuls (K=64, partition offsets 0/64) into one
    [128, 1024] psum tile; diagonal blocks narrowed to the valid q
    range; one exp per k-tile over both heads (ACT, scale=1/8, per-k
    pad bias), fill-0 affine_select on the 128-wide partial triangle.
  - AV: out_unnorm[65, q] += V_aug.T @ PT over k-tiles; normalize in
    [d, q] layout: reciprocal of the ones-row sum, partition-broadcast
    (gpsimd DMA), one DVE multiply; `out` leaves the chip [c, s] and
    the host transposes it in the gather, like kct.
  - Emission order per q-slice: Q-projection of pair 0 first, then the
    full (non-diagonal) attention tiles -- they only need old kt/va --
    with the K projection, pair-1 Q and the V wave emitted behind them
    as PE filler while ACT chews the exp backlog; diagonal tiles and
    pair 1 follow.  The scalar engine then never idles more than the
    ~1.7us pair-0 Q projection at a slice boundary.
  - Inputs land on three DMA queues (x halves on sync+scalar, weights
    on gpsimd, wq first to match the Q-first emission); vc leaves on
    the gpsimd queue so mid-kernel stores never queue behind x loads.
  - PE warm-up runs off a gpsimd-memset tile so it starts ~6.3us in,
    independent of any DMA; heater matmuls inside the last (ACT-bound)
    pair keep HAM from re-throttling the PE near the end.
"""

import numpy as np

P = 128
S = 2048
HIN = 1024
C = 256  # columns per core = 4 heads * 64
HD = 64
NCORES = 8
HC = HIN // P  # 8 contraction chunks
NKT = S // P  # 16 k-tiles
QW = 512  # q-slice width
NQ = S // QW  # 4 q-slices
NPAIR = C // P  # 2 head-pairs per core

_nc_cache = None


def build_nc():
    import concourse.bacc as bacc
    import concourse.mybir as mybir
    from concourse.tile import TileContext
    from contextlib import ExitStack

    f32 = mybir.dt.float32
    f32r = mybir.dt.float32r
    bf16 = mybir.dt.bfloat16
    Exp = mybir.ActivationFunctionType.Exp

    nc = bacc.Bacc(None, target_bir_lowering=False)

    # x and weights arrive partition-major (host pre-arranged) so every
    # DMA descriptor moves a contiguous 4KB run per partition
    xt = nc.declare_dram_parameter("xt", [4, 2, P, (HC // 2) * (S // 4)], bf16,
                                   isOutput=False)
    wq = nc.declare_dram_parameter("wq", [P, HC * C], bf16, isOutput=False)
    wk = nc.declare_dram_parameter("wk", [P, HC * C], bf16, isOutput=False)
    wv = nc.declare_dram_parameter("wv", [P, HC * C], bf16, isOutput=False)
    # small per-partition constants are PACKED into two tensors (f32:
    # bqc|bkc|padneg, bf16: tri|ones) -- 7 fragmented 8-64B/partition
    # DMAs measured ~10GB/s on the scalar HWDGE queue and gated the
    # first exp/AV by several us
    cpkf = nc.declare_dram_parameter("cpkf", [P, 2 * NPAIR + NKT], f32,
                                     isOutput=False)
    cpkb = nc.declare_dram_parameter("cpkb", [P, 2 * P + C], bf16,
                                     isOutput=False)
    bv = nc.declare_dram_parameter("bv", [1, C], bf16, isOutput=False)
    onesr = nc.declare_dram_parameter("onesr", [1, HD], f32r, isOutput=False)
    out = nc.declare_dram_parameter("out", [C, S], bf16, isOutput=True)
    kct = nc.declare_dram_parameter("kct", [C, S], bf16, isOutput=True)
    vc = nc.declare_dram_parameter("vc", [S, C], bf16, isOutput=True)

    with TileContext(nc) as tc, ExitStack() as ctx:
        persist = ctx.enter_context(tc.tile_pool(name="persist", bufs=1))
        xt_sb = persist.tile([P, HC, S], bf16)
        wq_sb = persist.tile([P, HC, C], bf16)
        wk_sb = persist.tile([P, HC, C], bf16)
        wv_sb = persist.tile([P, HC, C], bf16)
        cpkf_sb = persist.tile([P, 2 * NPAIR + NKT], f32)
        bqc_sb = cpkf_sb[:, 0:NPAIR]
        bkc_sb = cpkf_sb[:, NPAIR : 2 * NPAIR]
        pn_sb = cpkf_sb[:, 2 * NPAIR : 2 * NPAIR + NKT]
        cpkb_sb = persist.tile([P, 2 * P + C], bf16)
        tri_sb = cpkb_sb[:, 0 : 2 * P].rearrange("p (h b) -> p h b", h=2)
        ones_sb = cpkb_sb[:, 2 * P : 2 * P + C]
        bv_sb = persist.tile([1, C], bf16)
        warm_sb = persist.tile([HD, NKT], f32)
        ones_r = persist.tile([1, HD], f32r)
        qt_bf = persist.tile([P, NPAIR, S], bf16)
        kt_sb = persist.tile([P, NPAIR, S], bf16)
        va_bf = persist.tile([P, NKT, NPAIR, 2 * (HD + 1)], bf16)
        heat_sb = persist.tile([P, P + QW], bf16)

        # the heater tile is DVE-memset (vector idles at startup), so the
        # PE warm-up has no DMA or gpsimd dependency at all
        nc.vector.memset(heat_sb[:], 0)
        quarter = S // 4
        half = HC // 2

        def x_piece(eng, h, g):
            eng.dma_start(
                xt_sb[:, g * half : (g + 1) * half,
                      h * quarter : (h + 1) * quarter],
                xt[h, g, :, :].rearrange("p (a s) -> p a s", a=half),
            )

        # packed constants on the scalar queue (each dma_start costs
        # ~0.65us of descriptor-gen on its issuing engine), then the g=1
        # half of quarter 0, wv, and quarters 2-3; sync carries quarter-0
        # g=0 and quarter 1 (it later carries out/kct), gpsimd the wk/wq
        # halves (it later carries vc) -- three queues in parallel, with
        # everything qi=0 needs (quarter 0, wk, wq, wv) landed by ~17us
        nc.scalar.dma_start(cpkf_sb[:], cpkf[:])
        nc.scalar.dma_start(cpkb_sb[:], cpkb[:])
        nc.scalar.dma_start(bv_sb[:], bv[:])
        nc.scalar.dma_start(ones_r[:], onesr[:])
        nc.scalar.dma_start(
            wv_sb[:, :, :], wv[:, :].rearrange("p (a c) -> p a c", a=HC)
        )
        # wk lands first (the qi=0 emission is K-first: everything at
        # qi=0 needs kt), split in chunk-halves so the j0-3 projections
        # start as soon as the first half arrives
        for w_sb, w in ((wk_sb, wk), (wq_sb, wq)):
            for g in range(2):
                nc.gpsimd.dma_start(
                    w_sb[:, g * half : (g + 1) * half, :],
                    w[:, g * half * C : (g + 1) * half * C]
                    .rearrange("p (a c) -> p a c", a=half),
                )
        # all of x on the (fast) sync queue, quarter-major: the scalar
        # HWDGE queue measured ~3-4x slower, so it only carries the small
        # consts and wv
        for h in range(4):
            for g in range(2):
                x_piece(nc.sync, h, g)
        # dummy broadcast: loads the gpsimd PartitionBroadcast ucode
        # library (~7us) during the startup DMA wait instead of at the
        # first normalize
        nc.gpsimd.partition_broadcast(warm_sb[:], pn_sb[0:1, :])
        # ones columns of V_aug (positions 64 and 129).  Written by DVE, not
        # DMA: a strided sub-word DMA write would RMW-race the adjacent
        # DVE-written V columns.
        ones3 = ones_sb[:, : NKT * NPAIR].rearrange("p (a b) -> p a b", a=NKT)
        nc.vector.tensor_copy(
            out=va_bf[:, :, :, HD : HD + 1], in_=ones3[:, :, :, None]
        )
        nc.vector.tensor_copy(
            out=va_bf[:, :, :, 2 * HD + 1 : 2 * HD + 2], in_=ones3[:, :, :, None]
        )

        psum = ctx.enter_context(tc.tile_pool(name="psum", bufs=2, space="PSUM"))
        work = ctx.enter_context(tc.tile_pool(name="work", bufs=3))

        # p-state warm-up: the PE reaches full clock during the input DMA
        # wait; the results are never read.  The upfront heaters write
        # the av-pool slot (first real AV write is ~22us in), NOT the
        # proj pool -- a proj slot target would serialize the projection
        # chains against every heater the scheduler spreads into a
        # DMA-wait gap, and an st slot would halve the scores->exp
        # pipeline depth.  The late (qi=3) heaters use a proj-pool slot,
        # idle by then.
        heat_ps = psum.tile([HD + 1, 2, QW], f32, tag="av", bufs=1,
                            name="heat_ps")
        warm_ps = psum.tile([P, QW], f32, tag="proj", bufs=2, name="warm_ps")

        def heater(n=1, late=False):
            for _ in range(n):
                if late:
                    nc.tensor.matmul(
                        warm_ps, heat_sb[:, 0:P], heat_sb[:, P : P + QW],
                        start=True, stop=True,
                    )
                else:
                    nc.tensor.matmul(
                        heat_ps[:, 0, :], heat_sb[:, 0 : HD + 1],
                        heat_sb[:, P : P + QW], start=True, stop=True,
                    )

        heater(26)

        def proj_slice(qi, w_sb, b_sb, dst, pairs=(0, 1), evict="vector"):
            # project one q/k-slice of K or Q for the given pairs; for
            # qi=0 the two x chunk-halves are in flight on separate
            # queues, so run j 0-3 before j 4-7 (PE consumes half 0
            # while half 1 lands).  evict="scalar" runs the bias
            # eviction on the (idle-at-phase-boundary) scalar engine
            # instead of queueing behind the DVE's normalize backlog.
            qsl = slice(qi * QW, (qi + 1) * QW)
            jws = ((0, 4), (4, 8)) if qi == 0 else ((0, 8),)
            for p in pairs:
                csl = slice(p * P, (p + 1) * P)
                ps = psum.tile([P, QW], f32, tag="proj", bufs=2, name="p_ps")
                for j0, j1 in jws:
                    for j in range(j0, j1):
                        nc.tensor.matmul(
                            ps, w_sb[:, j, csl], xt_sb[:, j, qsl],
                            start=(j == 0), stop=(j == HC - 1),
                        )
                if evict == "scalar":
                    nc.scalar.add(dst[:, p, qsl], ps, b_sb[:, p : p + 1])
                else:
                    nc.vector.tensor_scalar_add(
                        dst[:, p, qsl], ps, b_sb[:, p : p + 1]
                    )

        def v_tile(i):
            ksl = slice(i * P, (i + 1) * P)
            ps = psum.tile([P, QW], f32, tag="proj", bufs=2, name="v_ps")[:, :C]
            for j in range(HC):
                nc.tensor.matmul(
                    ps, xt_sb[:, j, ksl], wv_sb[:, j, :],
                    start=(j == 0), stop=False,
                )
            nc.tensor.matmul(
                ps, ones_sb[:1, :P], bv_sb[:1, :], start=False, stop=True
            )
            sb = work.tile([P, C], bf16, tag="projsb", bufs=4, name="v_sb")
            # vector, NOT nc.any: the scheduler puts "any" copies on
            # the scalar engine, where each blocks the following exps
            # in the ACT FIFO while it waits for its V matmuls
            # (gpsimd cannot read PSUM)
            nc.vector.tensor_copy(out=sb[:], in_=ps)
            # vc leaves on the gpsimd queue so it never queues behind
            # the x loads on sync
            nc.gpsimd.dma_start(vc[ksl, :], sb[:])
            for p in range(NPAIR):
                nc.vector.tensor_copy(
                    out=va_bf[:, i, p, 0:HD], in_=sb[:, p * P : p * P + HD]
                )
                nc.vector.tensor_copy(
                    out=va_bf[:, i, p, HD + 1 : 2 * HD + 1],
                    in_=sb[:, p * P + HD : (p + 1) * P],
                )

        def tile_scores_exp(qi, p, t):
            # scores for both heads (concurrent row-tiled matmuls), exp,
            # and the diagonal-block mask; returns the pt tile + width
            ksl = slice(t * P, (t + 1) * P)
            d = t - 4 * qi
            W = QW if d < 0 else QW - d * P
            q0 = qi * QW + (0 if d < 0 else d * P)
            st = psum.tile([P, 2 * QW], f32, tag="st", bufs=2, name="st")
            nc.tensor.matmul(
                st[:, 0:W], kt_sb[0:HD, p, ksl],
                qt_bf[0:HD, p, q0 : q0 + W], start=True, stop=True,
            )
            nc.tensor.matmul(
                st[:, QW : QW + W], kt_sb[HD:P, p, ksl],
                qt_bf[HD:P, p, q0 : q0 + W], start=True, stop=True,
            )
            pt = work.tile([P, 2, QW], bf16, tag="pt", bufs=32, name="pt")
            st3 = st[:].rearrange("p (h w) -> p h w", h=2)[:, :, 0:W]
            nc.scalar.activation(
                pt[:, :, 0:W], st3, Exp, bias=pn_sb[:, t : t + 1],
                scale=0.125,
            )
            if d >= 0:
                nc.vector.tensor_mul(
                    pt[:, :, 0:P], pt[:, :, 0:P], tri_sb[:]
                )
            return pt, W

        def tile_av(av, pt, W, p, t, start, stop):
            nc.tensor.matmul(
                av[:, 0, QW - W :], va_bf[:, t, p, 0 : HD + 1],
                pt[:, 0, 0:W], start=start, stop=stop,
            )
            nc.tensor.matmul(
                av[:, 1, QW - W :], va_bf[:, t, p, HD + 1 : 2 * HD + 2],
                pt[:, 1, 0:W], start=start, stop=stop,
            )

        def attention_core(qi, p, trange=None, avs=None, heat_every=0):
            # one [65, 2, QW] psum tile holds BOTH heads' unnormalized AV
            # (2 adjacent banks) so normalize can run single merged
            # [1, 2*QW] ops instead of per-head ones
            av = avs if avs is not None else psum.tile(
                [HD + 1, 2, QW], f32, tag="av", bufs=1, name="av"
            )
            tmax = 4 * qi + 4
            for ti, t in enumerate(trange if trange is not None else range(tmax)):
                if heat_every and ti and ti % heat_every == 0:
                    heater(late=True)
                pt, W = tile_scores_exp(qi, p, t)
                tile_av(av, pt, W, p, t, start=(t == 0), stop=(t == tmax - 1))
            return av

        def normalize(qi, p, av, use_pe_outer):
            # normalize in [d, q] layout: reciprocal of the ones-row sum
            # (both heads in one [1, 2*QW] sweep), partition-broadcast
            # (gpsimd DMA; PE outer product on the last slice so gpsimd's
            # slow post-broadcast drain doesn't stretch the tail), one
            # DVE multiply per head; `out` leaves the chip [c, s] (the
            # host transposes it in the gather, like kct).
            # The whole av tile is copied to SBUF in ONE op first: the
            # single av psum slot is WAR-shared with the next pair's
            # accumulation, and holding it through the ~5us
            # rcp->broadcast->mul latency chain cascades (via the pt
            # pool) into the next phase's exps.
            if not use_pe_outer:
                avc = work.tile([HD + 1, 2, QW], f32, tag="avc", bufs=2,
                                name="avc")
                nc.vector.tensor_copy(out=avc[:], in_=av[:])
                av = avc
            dsb = work.tile([1, 2 * QW], f32, tag="dsb", bufs=2, name="dsb")
            dsb3 = dsb[:].rearrange("p (h w) -> p h w", h=2)
            if use_pe_outer:
                nc.scalar.copy(dsb3, av[HD : HD + 1, :, :])
            else:
                nc.vector.tensor_copy(out=dsb3, in_=av[HD : HD + 1, :, :])
            rcp = work.tile([1, 2 * QW], f32, tag="rcp", bufs=2, name="rcp")
            nc.vector.reciprocal_approx_fast(rcp[:], dsb[:])
            rb_sb = work.tile([HD, 2 * QW], f32, tag="rb", bufs=2, name="rb")
            if not use_pe_outer:
                nc.gpsimd.partition_broadcast(rb_sb[:], rcp[:])
            else:
                rcp_r = work.tile([1, 2 * QW], f32r, tag="rcpr", bufs=2,
                                  name="rcpr")
                nc.vector.tensor_copy(out=rcp_r[:], in_=rcp[:])
                rb_ps = psum.tile([P, 2 * QW], f32, tag="st", bufs=2,
                                  name="rb_ps")[0:HD, :]
                for h in range(2):  # f32 moving operand caps at 512
                    nc.tensor.matmul(
                        rb_ps[:, h * QW : (h + 1) * QW], ones_r[:],
                        rcp_r[:, h * QW : (h + 1) * QW],
                        start=True, stop=True,
                    )
                nc.vector.tensor_copy(out=rb_sb[:], in_=rb_ps)
            rb3 = rb_sb[:].rearrange("p (h w) -> p h w", h=2)
            for h in range(2):
                osb = work.tile([HD, QW], bf16, tag="osb", bufs=3, name="osb")
                nc.vector.tensor_mul(osb[:], av[0:HD, h, :], rb3[:, h, :])
                col = p * P + h * HD
                nc.sync.dma_start(
                    out[col : col + HD, qi * QW : (qi + 1) * QW], osb[:]
                )

        # Q-first emission: the full (non-diagonal) attention tiles of a
        # slice only need the new Q projection (their kt/va slices are
        # older), so they start ~1.7us into the phase; the K projection,
        # pair-1 Q and the V wave trail behind them as PE filler while
        # the scalar engine drains the exp backlog.  Pair 1's normalize
        # is deferred past the next slice's Q projection, and the very
        # last pair normalizes via the PE outer product (the PE is idle
        # by then) so gpsimd's slow post-broadcast drain starts well
        # before the end.
        # Software-pipelined emission: ALL of a slice's scores+exp are
        # emitted before its PE fillers (V tiles, AV sweeps), buffered in
        # the deep pt pool, and the NEXT slice's pair-0 full scores+exp
        # are hoisted before this slice's pair-1 AV sweep -- the exp
        # stream on the scalar engine then never waits on filler blocks.
        # Pair-1 normalize is deferred past the next slice's head, and
        # the very last pair normalizes via the PE outer product (the PE
        # is idle by then) so gpsimd's slow post-broadcast drain starts
        # well before the end.
        pending = None
        proj_slice(0, wk_sb, bkc_sb, kt_sb)
        proj_slice(0, wq_sb, bqc_sb, qt_bf)
        full0_pts = []
        diag0_pts = [(t,) + tile_scores_exp(0, 0, t) for t in range(4)]
        for qi in range(NQ):
            if pending is not None:
                normalize(qi - 1, 1, pending, use_pe_outer=False)
            av0 = psum.tile([HD + 1, 2, QW], f32, tag="av", bufs=1, name="av")
            for t, pt, W in full0_pts:
                tile_av(av0, pt, W, 0, t, start=(t == 0), stop=False)
            # pair 1's scores+exp: all inputs were projected a phase ago
            pts1 = []
            for t in range(4 * qi + 4):
                pts1.append((t,) + tile_scores_exp(qi, 1, t))
            for t, pt, W in diag0_pts:
                v_tile(t)
                tile_av(av0, pt, W, 0, t,
                        start=(t == 0 and qi == 0),
                        stop=(t == 4 * qi + 3))
            normalize(qi, 0, av0, use_pe_outer=False)
            if qi < NQ - 1:
                # hoist ALL of the next slice's projections and its
                # pair-0 scores+exp into this phase's tail: the next
                # phase then starts with its whole exp stream unblocked
                proj_slice(qi + 1, wq_sb, bqc_sb, qt_bf, pairs=(0,),
                           evict="scalar")
                full0_pts = [
                    (t,) + tile_scores_exp(qi + 1, 0, t)
                    for t in range(4 * (qi + 1))
                ]
                proj_slice(qi + 1, wk_sb, bkc_sb, kt_sb)
                proj_slice(qi + 1, wq_sb, bqc_sb, qt_bf, pairs=(1,),
                           evict="scalar")
                diag0_pts = [
                    (t,) + tile_scores_exp(qi + 1, 0, t)
                    for t in range(4 * (qi + 1), 4 * (qi + 1) + 4)
                ]
            av1 = psum.tile([HD + 1, 2, QW], f32, tag="av", bufs=1, name="av")
            for ti, (t, pt, W) in enumerate(pts1):
                if qi == NQ - 1 and ti:
                    heater(late=True)
                tile_av(av1, pt, W, 1, t,
                        start=(t == 0), stop=(t == 4 * qi + 3))
            pending = av1
            # K_cache leaves the chip in kt's [c, s] layout (contiguous
            # DMA); the host transposes it during the gather
            nc.sync.dma_start(
                kct[:, qi * QW : (qi + 1) * QW]
                .rearrange("(a p) s -> p a s", p=P),
                kt_sb[:, :, qi * QW : (qi + 1) * QW],
            )
        normalize(NQ - 1, 1, pending, use_pe_outer=True)

    nc.finalize()
    return nc


def get_nc():
    global _nc_cache
    if _nc_cache is None:
        _nc_cache = build_nc()
    return _nc_cache


def _w_pre(Wslice):
    # [HIN, C] -> partition-major [P, HC*C]: per partition p a contiguous
    # 4KB run holding its rows of every contraction chunk
    return np.ascontiguousarray(
        Wslice.reshape(HC, P, C).transpose(1, 0, 2).reshape(P, HC * C)
    )


def _x_pre(xT):
    # [HIN, S] -> [4(h), 2(g), P, 4*512]: per (quarter h, chunk-half g,
    # partition p) a contiguous 4KB run
    a = xT.reshape(2, 4, P, 4, QW)  # [g, jm, p, h, s]
    return np.ascontiguousarray(
        a.transpose(3, 0, 2, 1, 4).reshape(4, 2, P, 4 * QW)
    )


def make_in_maps(x, pad_mask, Wq, bq, Wk, bk, Wv, bv):
    from ml_dtypes import bfloat16

    x = np.asarray(x, np.float32)
    pad_mask = np.asarray(pad_mask, np.float32)
    Wq = np.asarray(Wq, bfloat16)
    bq = np.asarray(bq, np.float32)
    Wk = np.asarray(Wk, bfloat16)
    bk = np.asarray(bk, np.float32)
    Wv = np.asarray(Wv, bfloat16)
    bv = np.asarray(bv, bfloat16)
    xts = [_x_pre(x[b].T.astype(bfloat16)) for b in range(2)]
    in_maps = []
    for c in range(NCORES):
        b, g = divmod(c, 4)
        cols = slice(g * C, (g + 1) * C)
        pn = ((pad_mask[b] - 1.0) * 1e6).reshape(NKT, P).T  # [P, NKT]
        trih = (np.arange(P)[None, :] >= np.arange(P)[:, None]).astype(bfloat16)
        cpkf = np.concatenate(
            [bq[cols].reshape(NPAIR, P).T, bk[cols].reshape(NPAIR, P).T, pn],
            axis=1,
        ).astype(np.float32)
        cpkb = np.concatenate(
            [trih, trih, np.ones((P, C), bfloat16)], axis=1
        ).astype(bfloat16)
        in_maps.append(
            dict(
                xt=xts[b],
                cpkf=np.ascontiguousarray(cpkf),
                cpkb=np.ascontiguousarray(cpkb),
                onesr=np.ones((1, HD), np.float32),
                wq=_w_pre(Wq[:, cols]),
                wk=_w_pre(Wk[:, cols]),
                wv=_w_pre(Wv[:, cols]),
                bv=np.ascontiguousarray(bv[cols].reshape(1, C)),
            )
        )
    return in_maps


def gather(results):
    B = 2
    out = np.empty((B, S, HIN), np.float32)
    kcache = np.empty((B, S, HIN), np.float32)
    vcache = np.empty((B, S, HIN), np.float32)
    for c in range(NCORES):
        b, g = divmod(c, 4)
        cols = slice(g * C, (g + 1) * C)
        out[b, :, cols] = results[c]["out"].T
        kcache[b, :, cols] = results[c]["kct"].T
        vcache[b, :, cols] = results[c]["vc"]
    return out, kcache, vcache


def kernel(x, pad_mask, Wq, bq, Wk, bk, Wv, bv):
    from concourse.bass_utils import run_bass_kernel_spmd

    nc = get_nc()
    in_maps = make_in_maps(x, pad_mask, Wq, bq, Wk, bk, Wv, bv)
    res = run_bass_kernel_spmd(nc, in_maps, list(range(NCORES)))
    return gather(res.results)


# revision 33
# speedup vs baseline: 1.0269x; 1.0206x over previous
"""Causal multi-head attention (16 heads, hd=64) on 8 trn2 NeuronCores.

Sharding: core c -> batch b = c // 4, head-group g = c % 4 (4 heads = 256
columns of Wq/Wk/Wv).  Each core computes its [S, 256] slice of the three
outputs (attn out, K_cache, V_cache); the host gathers slices.

Per-core pipeline (Tile framework), all matmuls in bf16 (f32 PSUM
accumulation, ~4e-3 relative error vs the f32 reference):
  - xT [1024, S] is host-transposed x[b]; weights/biases host-sliced.
  - KT/QT [c, q] computed directly (lhsT = W chunk), per-partition bias
    added during the DVE eviction; K_cache leaves the chip in kt's [c, s]
    layout (contiguous DMA) and the host transposes it in the gather.
  - Vf natural [s, c] (rank-1 bias matmul) -> V_cache + V_aug tiles
    [k, 65] per head (ones column -> softmax denominator; ones written
    by DVE -- a strided sub-word DMA would RMW-race adjacent columns).
  - scores ST[k, q]: the two heads of a pair run as concurrent
    row-tiled matmuls (K=64, partition offsets 0/64) into one
    [128, 1024] psum tile; diagonal blocks narrowed to the valid q
    range; one exp per k-tile over both heads (ACT, scale=1/8, per-k
    pad bias), fill-0 affine_select on the 128-wide partial triangle.
  - AV: out_unnorm[65, q] += V_aug.T @ PT over k-tiles; normalize in
    [d, q] layout: reciprocal of the ones-row sum, partition-broadcast
    (gpsimd DMA), one DVE multiply; `out` leaves the chip [c, s] and
    the host transposes it in the gather, like kct.
  - Emission order per q-slice: Q-projection of pair 0 first, then the
    full (non-diagonal) attention tiles -- they only need old kt/va --
    with the K projection, pair-1 Q and the V wave emitted behind them
    as PE filler while ACT chews the exp backlog; diagonal tiles and
    pair 1 follow.  The scalar engine then never idles more than the
    ~1.7us pair-0 Q projection at a slice boundary.
  - Inputs land on three DMA queues (x halves on sync+scalar, weights
    on gpsimd, wq first to match the Q-first emission); vc leaves on
    the gpsimd queue so mid-kernel stores never queue behind x loads.
  - PE warm-up runs off a gpsimd-memset tile so it starts ~6.3us in,
    independent of any DMA; heater matmuls inside the last (ACT-bound)
    pair keep HAM from re-throttling the PE near the end.
"""

import numpy as np

P = 128
S = 2048
HIN = 1024
C = 256  # columns per core = 4 heads * 64
HD = 64
NCORES = 8
HC = HIN // P  # 8 contraction chunks
NKT = S // P  # 16 k-tiles
QW = 512  # q-slice width
NQ = S // QW  # 4 q-slices
NPAIR = C // P  # 2 head-pairs per core

_nc_cache = None


def build_nc():
    import concourse.bacc as bacc
    import concourse.mybir as mybir
    from concourse.tile import TileContext
    from contextlib import ExitStack

    f32 = mybir.dt.float32
    f32r = mybir.dt.float32r
    bf16 = mybir.dt.bfloat16
    Exp = mybir.ActivationFunctionType.Exp

    nc = bacc.Bacc(None, target_bir_lowering=False)

    # x and weights arrive partition-major (host pre-arranged) so every
    # DMA descriptor moves a contiguous 4KB run per partition
    xt = nc.declare_dram_parameter("xt", [4, 2, P, (HC // 2) * (S // 4)], bf16,
                                   isOutput=False)
    wq = nc.declare_dram_parameter("wq", [P, HC * C], bf16, isOutput=False)
    wk = nc.declare_dram_parameter("wk", [P, HC * C], bf16, isOutput=False)
    wv = nc.declare_dram_parameter("wv", [P, HC * C], bf16, isOutput=False)
    # small per-partition constants are PACKED into two tensors (f32:
    # bqc|bkc|padneg, bf16: tri|ones) -- 7 fragmented 8-64B/partition
    # DMAs measured ~10GB/s on the scalar HWDGE queue and gated the
    # first exp/AV by several us
    cpkf = nc.declare_dram_parameter("cpkf", [P, 2 * NPAIR + NKT], f32,
                                     isOutput=False)
    cpkb = nc.declare_dram_parameter("cpkb", [P, 2 * P + C], bf16,
                                     isOutput=False)
    bv = nc.declare_dram_parameter("bv", [1, C], bf16, isOutput=False)
    onesr = nc.declare_dram_parameter("onesr", [1, HD], f32r, isOutput=False)
    out = nc.declare_dram_parameter("out", [C, S], bf16, isOutput=True)
    kct = nc.declare_dram_parameter("kct", [C, S], bf16, isOutput=True)
    vc = nc.declare_dram_parameter("vc", [S, C], bf16, isOutput=True)

    with TileContext(nc) as tc, ExitStack() as ctx:
        persist = ctx.enter_context(tc.tile_pool(name="persist", bufs=1))
        xt_sb = persist.tile([P, HC, S], bf16)
        wq_sb = persist.tile([P, HC, C], bf16)
        wk_sb = persist.tile([P, HC, C], bf16)
        wv_sb = persist.tile([P, HC, C], bf16)
        cpkf_sb = persist.tile([P, 2 * NPAIR + NKT], f32)
        bqc_sb = cpkf_sb[:, 0:NPAIR]
        bkc_sb = cpkf_sb[:, NPAIR : 2 * NPAIR]
        pn_sb = cpkf_sb[:, 2 * NPAIR : 2 * NPAIR + NKT]
        cpkb_sb = persist.tile([P, 2 * P + C], bf16)
        tri_sb = cpkb_sb[:, 0 : 2 * P].rearrange("p (h b) -> p h b", h=2)
        ones_sb = cpkb_sb[:, 2 * P : 2 * P + C]
        bv_sb = persist.tile([1, C], bf16)
        warm_sb = persist.tile([HD, NKT], f32)
        ones_r = persist.tile([1, HD], f32r)
        qt_bf = persist.tile([P, NPAIR, S], bf16)
        kt_sb = persist.tile([P, NPAIR, S], bf16)
        va_bf = persist.tile([P, NKT, NPAIR, 2 * (HD + 1)], bf16)
        heat_sb = persist.tile([P, P + QW], bf16)

        # the heater tile is DVE-memset (vector idles at startup), so the
        # PE warm-up has no DMA or gpsimd dependency at all
        nc.vector.memset(heat_sb[:], 0)
        quarter = S // 4
        half = HC // 2

        def x_piece(eng, h, g):
            eng.dma_start(
                xt_sb[:, g * half : (g + 1) * half,
                      h * quarter : (h + 1) * quarter],
                xt[h, g, :, :].rearrange("p (a s) -> p a s", a=half),
            )

        # packed constants on the scalar queue (each dma_start costs
        # ~0.65us of descriptor-gen on its issuing engine), then the g=1
        # half of quarter 0, wv, and quarters 2-3; sync carries quarter-0
        # g=0 and quarter 1 (it later carries out/kct), gpsimd the wk/wq
        # halves (it later carries vc) -- three queues in parallel, with
        # everything qi=0 needs (quarter 0, wk, wq, wv) landed by ~17us
        nc.scalar.dma_start(cpkf_sb[:], cpkf[:])
        nc.scalar.dma_start(cpkb_sb[:], cpkb[:])
        nc.scalar.dma_start(bv_sb[:], bv[:])
        nc.scalar.dma_start(ones_r[:], onesr[:])
        nc.scalar.dma_start(
            wv_sb[:, :, :], wv[:, :].rearrange("p (a c) -> p a c", a=HC)
        )
        # wk lands first (the qi=0 emission is K-first: everything at
        # qi=0 needs kt), split in chunk-halves so the j0-3 projections
        # start as soon as the first half arrives
        for w_sb, w in ((wk_sb, wk), (wq_sb, wq)):
            for g in range(2):
                nc.gpsimd.dma_start(
                    w_sb[:, g * half : (g + 1) * half, :],
                    w[:, g * half * C : (g + 1) * half * C]
                    .rearrange("p (a c) -> p a c", a=half),
                )
        # all of x on the (fast) sync queue, quarter-major: the scalar
        # HWDGE queue measured ~3-4x slower, so it only carries the small
        # consts and wv
        for h in range(4):
            for g in range(2):
                x_piece(nc.sync, h, g)
        # dummy broadcast: loads the gpsimd PartitionBroadcast ucode
        # library (~7us) during the startup DMA wait instead of at the
        # first normalize
        nc.gpsimd.partition_broadcast(warm_sb[:], pn_sb[0:1, :])
        # ones columns of V_aug (positions 64 and 129).  Written by DVE, not
        # DMA: a strided sub-word DMA write would RMW-race the adjacent
        # DVE-written V columns.
        ones3 = ones_sb[:, : NKT * NPAIR].rearrange("p (a b) -> p a b", a=NKT)
        nc.vector.tensor_copy(
            out=va_bf[:, :, :, HD : HD + 1], in_=ones3[:, :, :, None]
        )
        nc.vector.tensor_copy(
            out=va_bf[:, :, :, 2 * HD + 1 : 2 * HD + 2], in_=ones3[:, :, :, None]
        )

        psum = ctx.enter_context(tc.tile_pool(name="psum", bufs=2, space="PSUM"))
        work = ctx.enter_context(tc.tile_pool(name="work", bufs=3))

        # p-state warm-up: the PE reaches full clock during the input DMA
        # wait; the results are never read.  The upfront heaters write
        # the av-pool slot (first real AV write is ~22us in), NOT the
        # proj pool -- a proj slot target would serialize the projection
        # chains against every heater the scheduler spreads into a
        # DMA-wait gap, and an st slot would halve the scores->exp
        # pipeline depth.  The late (qi=3) heaters use a proj-pool slot,
        # idle by then.
        heat_ps = psum.tile([HD + 1, 2, QW], f32, tag="av", bufs=1,
                            name="heat_ps")
        warm_ps = psum.tile([P, QW], f32, tag="proj", bufs=2, name="warm_ps")

        def heater(n=1, late=False):
            for _ in range(n):
                if late:
                    nc.tensor.matmul(
                        warm_ps, heat_sb[:, 0:P], heat_sb[:, P : P + QW],
                        start=True, stop=True,
                    )
                else:
                    nc.tensor.matmul(
                        heat_ps[:, 0, :], heat_sb[:, 0 : HD + 1],
                        heat_sb[:, P : P + QW], start=True, stop=True,
                    )

        heater(16)

        def proj_slice(qi, w_sb, b_sb, dst, pairs=(0, 1), evict="vector"):
            # project one q/k-slice of K or Q for the given pairs; for
            # qi=0 the two x chunk-halves are in flight on separate
            # queues, so run j 0-3 before j 4-7 (PE consumes half 0
            # while half 1 lands).  evict="scalar" runs the bias
            # eviction on the (idle-at-phase-boundary) scalar engine
            # instead of queueing behind the DVE's normalize backlog.
            qsl = slice(qi * QW, (qi + 1) * QW)
            jws = ((0, 4), (4, 8)) if qi == 0 else ((0, 8),)
            for p in pairs:
                csl = slice(p * P, (p + 1) * P)
                ps = psum.tile([P, QW], f32, tag="proj", bufs=2, name="p_ps")
                for j0, j1 in jws:
                    for j in range(j0, j1):
                        nc.tensor.matmul(
                            ps, w_sb[:, j, csl], xt_sb[:, j, qsl],
                            start=(j == 0), stop=(j == HC - 1),
                        )
                if evict == "scalar":
                    nc.scalar.add(dst[:, p, qsl], ps, b_sb[:, p : p + 1])
                else:
                    nc.vector.tensor_scalar_add(
                        dst[:, p, qsl], ps, b_sb[:, p : p + 1]
                    )

        def v_tile(i):
            ksl = slice(i * P, (i + 1) * P)
            ps = psum.tile([P, QW], f32, tag="proj", bufs=2, name="v_ps")[:, :C]
            for j in range(HC):
                nc.tensor.matmul(
                    ps, xt_sb[:, j, ksl], wv_sb[:, j, :],
                    start=(j == 0), stop=False,
                )
            nc.tensor.matmul(
                ps, ones_sb[:1, :P], bv_sb[:1, :], start=False, stop=True
            )
            sb = work.tile([P, C], bf16, tag="projsb", bufs=4, name="v_sb")
            # vector, NOT nc.any: the scheduler puts "any" copies on
            # the scalar engine, where each blocks the following exps
            # in the ACT FIFO while it waits for its V matmuls
            # (gpsimd cannot read PSUM)
            nc.vector.tensor_copy(out=sb[:], in_=ps)
            # vc leaves on the gpsimd queue so it never queues behind
            # the x loads on sync
            nc.gpsimd.dma_start(vc[ksl, :], sb[:])
            for p in range(NPAIR):
                nc.vector.tensor_copy(
                    out=va_bf[:, i, p, 0:HD], in_=sb[:, p * P : p * P + HD]
                )
                nc.vector.tensor_copy(
                    out=va_bf[:, i, p, HD + 1 : 2 * HD + 1],
                    in_=sb[:, p * P + HD : (p + 1) * P],
                )

        def tile_scores_exp(qi, p, t):
            # scores for both heads (concurrent row-tiled matmuls), exp,
            # and the diagonal-block mask; returns the pt tile + width
            ksl = slice(t * P, (t + 1) * P)
            d = t - 4 * qi
            W = QW if d < 0 else QW - d * P
            q0 = qi * QW + (0 if d < 0 else d * P)
            st = psum.tile([P, 2 * QW], f32, tag="st", bufs=2, name="st")
            nc.tensor.matmul(
                st[:, 0:W], kt_sb[0:HD, p, ksl],
                qt_bf[0:HD, p, q0 : q0 + W], start=True, stop=True,
            )
            nc.tensor.matmul(
                st[:, QW : QW + W], kt_sb[HD:P, p, ksl],
                qt_bf[HD:P, p, q0 : q0 + W], start=True, stop=True,
            )
            pt = work.tile([P, 2, QW], bf16, tag="pt", bufs=20, name="pt")
            st3 = st[:].rearrange("p (h w) -> p h w", h=2)[:, :, 0:W]
            nc.scalar.activation(
                pt[:, :, 0:W], st3, Exp, bias=pn_sb[:, t : t + 1],
                scale=0.125,
            )
            if d >= 0:
                nc.vector.tensor_mul(
                    pt[:, :, 0:P], pt[:, :, 0:P], tri_sb[:]
                )
            return pt, W

        def tile_av(av, pt, W, p, t, start, stop):
            nc.tensor.matmul(
                av[:, 0, QW - W :], va_bf[:, t, p, 0 : HD + 1],
                pt[:, 0, 0:W], start=start, stop=stop,
            )
            nc.tensor.matmul(
                av[:, 1, QW - W :], va_bf[:, t, p, HD + 1 : 2 * HD + 2],
                pt[:, 1, 0:W], start=start, stop=stop,
            )

        def attention_core(qi, p, trange=None, avs=None, heat_every=0):
            # one [65, 2, QW] psum tile holds BOTH heads' unnormalized AV
            # (2 adjacent banks) so normalize can run single merged
            # [1, 2*QW] ops instead of per-head ones
            av = avs if avs is not None else psum.tile(
                [HD + 1, 2, QW], f32, tag="av", bufs=1, name="av"
            )
            tmax = 4 * qi + 4
            for ti, t in enumerate(trange if trange is not None else range(tmax)):
                if heat_every and ti and ti % heat_every == 0:
                    heater(late=True)
                pt, W = tile_scores_exp(qi, p, t)
                tile_av(av, pt, W, p, t, start=(t == 0), stop=(t == tmax - 1))
            return av

        def normalize(qi, p, av, use_pe_outer):
            # normalize in [d, q] layout: reciprocal of the ones-row sum
            # (both heads in one [1, 2*QW] sweep), partition-broadcast
            # (gpsimd DMA; PE outer product on the last slice so gpsimd's
            # slow post-broadcast drain doesn't stretch the tail), one
            # DVE multiply per head; `out` leaves the chip [c, s] (the
            # host transposes it in the gather, like kct).
            # The whole av tile is copied to SBUF in ONE op first: the
            # single av psum slot is WAR-shared with the next pair's
            # accumulation, and holding it through the ~5us
            # rcp->broadcast->mul latency chain cascades (via the pt
            # pool) into the next phase's exps.
            if not use_pe_outer:
                avc = work.tile([HD + 1, 2, QW], f32, tag="avc", bufs=2,
                                name="avc")
                nc.vector.tensor_copy(out=avc[:], in_=av[:])
                av = avc
            dsb = work.tile([1, 2 * QW], f32, tag="dsb", bufs=2, name="dsb")
            dsb3 = dsb[:].rearrange("p (h w) -> p h w", h=2)
            if use_pe_outer:
                nc.scalar.copy(dsb3, av[HD : HD + 1, :, :])
            else:
                nc.vector.tensor_copy(out=dsb3, in_=av[HD : HD + 1, :, :])
            rcp = work.tile([1, 2 * QW], f32, tag="rcp", bufs=2, name="rcp")
            nc.vector.reciprocal_approx_fast(rcp[:], dsb[:])
            rb_sb = work.tile([HD, 2 * QW], f32, tag="rb", bufs=2, name="rb")
            if not use_pe_outer:
                nc.gpsimd.partition_broadcast(rb_sb[:], rcp[:])
            else:
                rcp_r = work.tile([1, 2 * QW], f32r, tag="rcpr", bufs=2,
                                  name="rcpr")
                nc.vector.tensor_copy(out=rcp_r[:], in_=rcp[:])
                rb_ps = psum.tile([P, 2 * QW], f32, tag="st", bufs=2,
                                  name="rb_ps")[0:HD, :]
                for h in range(2):  # f32 moving operand caps at 512
                    nc.tensor.matmul(
                        rb_ps[:, h * QW : (h + 1) * QW], ones_r[:],
                        rcp_r[:, h * QW : (h + 1) * QW],
                        start=True, stop=True,
                    )
                nc.vector.tensor_copy(out=rb_sb[:], in_=rb_ps)
            rb3 = rb_sb[:].rearrange("p (h w) -> p h w", h=2)
            for h in range(2):
                osb = work.tile([HD, QW], bf16, tag="osb", bufs=3, name="osb")
                nc.vector.tensor_mul(osb[:], av[0:HD, h, :], rb3[:, h, :])
                col = p * P + h * HD
                nc.sync.dma_start(
                    out[col : col + HD, qi * QW : (qi + 1) * QW], osb[:]
                )

        # Q-first emission: the full (non-diagonal) attention tiles of a
        # slice only need the new Q projection (their kt/va slices are
        # older), so they start ~1.7us into the phase; the K projection,
        # pair-1 Q and the V wave trail behind them as PE filler while
        # the scalar engine drains the exp backlog.  Pair 1's normalize
        # is deferred past the next slice's Q projection, and the very
        # last pair normalizes via the PE outer product (the PE is idle
        # by then) so gpsimd's slow post-broadcast drain starts well
        # before the end.
        # Software-pipelined emission: ALL of a slice's scores+exp are
        # emitted before its PE fillers (V tiles, AV sweeps), buffered in
        # the deep pt pool, and the NEXT slice's pair-0 full scores+exp
        # are hoisted before this slice's pair-1 AV sweep -- the exp
        # stream on the scalar engine then never waits on filler blocks.
        # Pair-1 normalize is deferred past the next slice's head, and
        # the very last pair normalizes via the PE outer product (the PE
        # is idle by then) so gpsimd's slow post-broadcast drain starts
        # well before the end.
        pending = None
        for qi in range(NQ):
            if qi == 0:
                # qi=0 has no full tiles; everything needs kt, so K first
                proj_slice(qi, wk_sb, bkc_sb, kt_sb)
                proj_slice(qi, wq_sb, bqc_sb, qt_bf, pairs=(0,))
            else:
                proj_slice(qi, wq_sb, bqc_sb, qt_bf, pairs=(0,),
                           evict="scalar")
            if pending is not None:
                normalize(qi - 1, 1, pending, use_pe_outer=False)
            av0 = attention_core(qi, 0, trange=range(0, 4 * qi))
            if qi > 0:
                proj_slice(qi, wk_sb, bkc_sb, kt_sb)
            proj_slice(qi, wq_sb, bqc_sb, qt_bf, pairs=(1,), evict="scalar")
            # ALL remaining scores+exp of the phase are emitted before the
            # PE filler (V tiles, AV sweeps): the deep pt pool buffers the
            # exp results, so ACT never waits on the fillers.  Pair 0's
            # diagonal AVs interleave after the single V tile each needs;
            # pair 1's AVs trail after normalize(qi,0) frees the av slot.
            pts0 = []
            for t in range(4 * qi, 4 * qi + 4):
                pts0.append((t,) + tile_scores_exp(qi, 0, t))
            pts1 = []
            for t in range(4 * qi + 4):
                pts1.append((t,) + tile_scores_exp(qi, 1, t))
            for t, pt, W in pts0:
                v_tile(t)
                tile_av(av0, pt, W, 0, t,
                        start=(t == 0), stop=(t == 4 * qi + 3))
            normalize(qi, 0, av0, use_pe_outer=False)
            av1 = psum.tile([HD + 1, 2, QW], f32, tag="av", bufs=1, name="av")
            for ti, (t, pt, W) in enumerate(pts1):
                if qi == NQ - 1 and ti:
                    heater(late=True)
                tile_av(av1, pt, W, 1, t,
                        start=(t == 0), stop=(t == 4 * qi + 3))
            pending = av1
            # K_cache leaves the chip in kt's [c, s] layout (contiguous
            # DMA); the host transposes it during the gather
            nc.sync.dma_start(
                kct[:, qi * QW : (qi + 1) * QW]
                .rearrange("(a p) s -> p a s", p=P),
                kt_sb[:, :, qi * QW : (qi + 1) * QW],
            )
        normalize(NQ - 1, 1, pending, use_pe_outer=True)

    nc.finalize()
    return nc


def get_nc():
    global _nc_cache
    if _nc_cache is None:
        _nc_cache = build_nc()
    return _nc_cache


def _w_pre(Wslice):
    # [HIN, C] -> partition-major [P, HC*C]: per partition p a contiguous
    # 4KB run holding its rows of every contraction chunk
    return np.ascontiguousarray(
        Wslice.reshape(HC, P, C).transpose(1, 0, 2).reshape(P, HC * C)
    )


def _x_pre(xT):
    # [HIN, S] -> [4(h), 2(g), P, 4*512]: per (quarter h, chunk-half g,
    # partition p) a contiguous 4KB run
    a = xT.reshape(2, 4, P, 4, QW)  # [g, jm, p, h, s]
    return np.ascontiguousarray(
        a.transpose(3, 0, 2, 1, 4).reshape(4, 2, P, 4 * QW)
    )


def make_in_maps(x, pad_mask, Wq, bq, Wk, bk, Wv, bv):
    from ml_dtypes import bfloat16

    x = np.asarray(x, np.float32)
    pad_mask = np.asarray(pad_mask, np.float32)
    Wq = np.asarray(Wq, bfloat16)
    bq = np.asarray(bq, np.float32)
    Wk = np.asarray(Wk, bfloat16)
    bk = np.asarray(bk, np.float32)
    Wv = np.asarray(Wv, bfloat16)
    bv = np.asarray(bv, bfloat16)
    xts = [_x_pre(x[b].T.astype(bfloat16)) for b in range(2)]
    in_maps = []
    for c in range(NCORES):
        b, g = divmod(c, 4)
        cols = slice(g * C, (g + 1) * C)
        pn = ((pad_mask[b] - 1.0) * 1e6).reshape(NKT, P).T  # [P, NKT]
        trih = (np.arange(P)[None, :] >= np.arange(P)[:, None]).astype(bfloat16)
        cpkf = np.concatenate(
            [bq[cols].reshape(NPAIR, P).T, bk[cols].reshape(NPAIR, P).T, pn],
            axis=1,
        ).astype(np.float32)
        cpkb = np.concatenate(
            [trih, trih, np.ones((P, C), bfloat16)], axis=1
        ).astype(bfloat16)
        in_maps.append(
            dict(
                xt=xts[b],
                cpkf=np.ascontiguousarray(cpkf),
                cpkb=np.ascontiguousarray(cpkb),
                onesr=np.ones((1, HD), np.float32),
                wq=_w_pre(Wq[:, cols]),
                wk=_w_pre(Wk[:, cols]),
                wv=_w_pre(Wv[:, cols]),
                bv=np.ascontiguousarray(bv[cols].reshape(1, C)),
            )
        )
    return in_maps


def gather(results):
    B = 2
    out = np.empty((B, S, HIN), np.float32)
    kcache = np.empty((B, S, HIN), np.float32)
    vcache = np.empty((B, S, HIN), np.float32)
    for c in range(NCORES):
        b, g = divmod(c, 4)
        cols = slice(g * C, (g + 1) * C)
        out[b, :, cols] = results[c]["out"].T
        kcache[b, :, cols] = results[c]["kct"].T
        vcache[b, :, cols] = results[c]["vc"]
    return out, kcache, vcache


def kernel(x, pad_mask, Wq, bq, Wk, bk, Wv, bv):
    from concourse.bass_utils import run_bass_kernel_spmd

    nc = get_nc()
    in_maps = make_in_maps(x, pad_mask, Wq, bq, Wk, bk, Wv, bv)
    res = run_bass_kernel_spmd(nc, in_maps, list(range(NCORES)))
    return gather(res.results)
